# revision 21
# baseline (speedup 1.0000x reference)
"""Multi-head self-attention (B=1, S=4096, D=2048, H=16, rotary_dim=64) on 8 TRN2 NeuronCores.

Head-sharded tensor parallelism: each core computes 2 heads end-to-end
(QKV projection + RoPE + full softmax attention) plus its slice of the
row-sharded output projection; the 8 partial [S, D] outputs are summed on
the host.

Precision: fp16 operands for all matmuls (PE accumulates in fp32; fp16
mantissa ~ matches the f32r/TF32 precision the PE offers for 4-byte
inputs, but gets fast 2-byte weight loads and DVE 2x modes). Softmax
skips max-subtraction (scores are ~N(0,1); exp is safe), computing
exp(s)/sum(exp(s)) directly with an fp32 PSUM denominator reduction.
"""

import numpy as np

import concourse.bass as bass
import concourse.mybir as mybir
import concourse.tile as tile
from concourse import bacc
from concourse.bass_utils import run_bass_kernel_spmd
from concourse.masks import make_identity

F32 = mybir.dt.float32
F32R = mybir.dt.float32r
FP16 = mybir.dt.float16
BF16 = mybir.dt.bfloat16

D = 2048
H = 16
HD = 128
ROT = 64
NCORES = 8
HPC = H // NCORES  # heads per core
SCALE = float(HD) ** -0.5

_CACHE = {}


def build_module(S=4096, ST=512, QTL=512):
    """Build the per-core SPMD bass module. Returns compiled nc.

    3-stage schedule to keep the PE busy across phases:
      A: QKV+RoPE for head 0
      B: attention(head0, qt) interleaved with QKV+RoPE(head1) s-tiles
      C: attention(head1, qt) interleaved with the output projection
    """
    NST = S // ST        # QKV s-tiles
    NKT = D // 128       # contraction tiles for QKV
    NQT = S // QTL       # attention q-tiles
    NKC = S // 128       # attention k-chunks
    ETL = 512            # out-proj e-tile
    NET = D // ETL
    assert NST == NQT

    nc = bacc.Bacc(None, target_bir_lowering=False, debug=True)

    xT_d = nc.dram_tensor("xT", [D, S], FP16, kind="ExternalInput")
    w_d = nc.dram_tensor("wsl", [D, 3 * HPC, 128], FP16, kind="ExternalInput")
    wo_d = nc.dram_tensor("wout", [HPC * HD, D], FP16, kind="ExternalInput")
    b_d = nc.dram_tensor("bsl", [128, 3 * HPC], F32, kind="ExternalInput")
    cs_d = nc.dram_tensor("cs", [ROT, 2, S], FP16, kind="ExternalInput")
    y_d = nc.dram_tensor("y", [S, D], F32, kind="ExternalOutput")

    xT_r = xT_d[:].rearrange("(t p) s -> p t s", p=128)
    w_r = w_d[:].rearrange("(t p) j m -> p t j m", p=128)
    wo_r = wo_d[:].rearrange("(t p) e -> p t e", p=128)

    with tile.TileContext(nc) as tc:
        with (
            tc.tile_pool(name="persist", bufs=1) as P,
            tc.tile_pool(name="wp", bufs=1) as wp,
            tc.tile_pool(name="xp", bufs=2) as xp,
            tc.tile_pool(name="csp", bufs=2) as csp,
            tc.tile_pool(name="vtp", bufs=2) as vtp,
            tc.tile_pool(name="rtp", bufs=2) as rtp,
            tc.tile_pool(name="ptp", bufs=5) as ptp,
            tc.tile_pool(name="accAp", bufs=2) as accAp,
            tc.tile_pool(name="accBp", bufs=2) as accBp,
            tc.tile_pool(name="rcp", bufs=2) as rcp,
            tc.tile_pool(name="yp", bufs=3) as yp,
            tc.tile_pool(name="ps1", bufs=2, space="PSUM") as ps1,
            tc.tile_pool(name="pst", bufs=1, space="PSUM") as pst,
            tc.tile_pool(name="pss", bufs=3, space="PSUM") as pssp,
            tc.tile_pool(name="pso", bufs=1, space="PSUM") as psop,
            tc.tile_pool(name="psd", bufs=1, space="PSUM") as psdp,
        ):
            QT = [P.tile([128, S], FP16, tag=f"qt{h}", name=f"qt{h}") for h in range(HPC)]
            KT = [P.tile([128, S], FP16, tag=f"kt{h}", name=f"kt{h}") for h in range(HPC)]
            V = [P.tile([128, NKC, 128], FP16, tag=f"v{h}", name=f"v{h}") for h in range(HPC)]
            AT = [P.tile([128, S], FP16, tag=f"at{h}", name=f"at{h}") for h in range(HPC)]
            b_sb = P.tile([128, 3 * HPC], F32)
            identr = P.tile([128, 128], FP16)
            ones_r = P.tile([128, 128], FP16)
            wo_sb = P.tile([128, HPC, D], FP16)
            w_sb = wp.tile([128, NKT, 3 * HPC, 128], FP16)
            nc.sync.dma_start(w_sb[:, 0 : NKT // 2, :, :], w_r[:, 0 : NKT // 2, :, :])
            nc.scalar.dma_start(w_sb[:, NKT // 2 :, :, :], w_r[:, NKT // 2 :, :, :])
            nc.gpsimd.dma_start(b_sb[:], b_d[:])
            make_identity(nc, identr)
            nc.vector.memset(ones_r[:], 1.0)

            # Warm the PE clock (HAM) during the initial weight/x DMAs: ~3.5us of
            # dummy matmuls so the first real matmuls run at 2.4 GHz.
            wm = psdp.tile([128, 128], F32, tag="dn", name="warm")
            for i in range(160):
                nc.tensor.matmul(wm[:], ones_r[:], ones_r[:],
                                 start=(i == 0), stop=(i == 159))
            wmr = rcp.tile([128, 1], F32, tag="rc", name="warmread")
            nc.vector.tensor_copy(wmr[:], wm[:, 0:1])

            def qkv_stile_thunks(h, st):
                """QKV projection + RoPE + V transpose for one head / s-tile,
                returned as a list of emission thunks for interleaving."""
                sl = bass.ts(st, ST)
                j = 3 * h
                state = {}
                thunks = []

                def t_load():
                    xt = xp.tile([128, NKT, ST], FP16, tag="xt", name=f"xt_{h}_{st}")
                    nc.sync.dma_start(xt[:, 0 : NKT // 2, :], xT_r[:, 0 : NKT // 2, sl])
                    nc.scalar.dma_start(xt[:, NKT // 2 :, :], xT_r[:, NKT // 2 :, sl])
                    cst = csp.tile([ROT, 2, ST], FP16, tag="cst", name=f"cst_{h}_{st}")
                    nc.sync.dma_start(cst[:], cs_d[:, :, sl])
                    state["xt"] = xt
                    state["cst"] = cst
                thunks.append(t_load)

                def t_group_open(which):
                    state[f"ps{which}"] = ps1.tile(
                        [128, ST], F32, tag="psqkv", name=f"ps{which}_{h}_{st}")
                def t_mms(which, k0, k1):
                    ps = state[f"ps{which}"]
                    xt = state["xt"]
                    for k in range(k0, k1):
                        nc.tensor.matmul(
                            ps[:], w_sb[:, k, j + which, :], xt[:, k, :],
                            start=(k == 0), stop=(k == NKT - 1),
                        )
                def t_evict_qk(which):
                    dst = QT[h] if which == 0 else KT[h]
                    cst = state["cst"]
                    nc.scalar.activation(
                        dst[:, sl], state[f"ps{which}"][:],
                        mybir.ActivationFunctionType.Identity,
                        bias=b_sb[:, j + which : j + which + 1],
                    )
                    tmp = rtp.tile([ROT, ST], FP16, tag="rtmp", name=f"rt_{h}_{st}_{which}")
                    nc.vector.tensor_copy(tmp[0 : ROT // 2, :], dst[ROT // 2 : ROT, sl])
                    nc.vector.tensor_copy(tmp[ROT // 2 : ROT, :], dst[0 : ROT // 2, sl])
                    nc.vector.tensor_mul(tmp[:], tmp[:], cst[:, 1, :])
                    nc.vector.tensor_mul(dst[0:ROT, sl], dst[0:ROT, sl], cst[:, 0, :])
                    nc.vector.tensor_add(dst[0:ROT, sl], dst[0:ROT, sl], tmp[:])
                def t_evict_v():
                    vt = vtp.tile([128, ST], FP16, tag="vt", name=f"vt_{h}_{st}")
                    nc.scalar.activation(
                        vt[:], state["ps2"][:], mybir.ActivationFunctionType.Identity,
                        bias=b_sb[:, j + 2 : j + 3],
                    )
                    state["vt"] = vt
                def t_vtr(sc):
                    ptr = pst.tile([128, 128], FP16, tag="ptr", name=f"ptr_{h}_{st}_{sc}")
                    nc.tensor.transpose(ptr[:], state["vt"][:, bass.ts(sc, 128)], identr[:])
                    nc.scalar.activation(V[h][:, st * (ST // 128) + sc, :], ptr[:],
                                         mybir.ActivationFunctionType.Copy)

                for which in range(3):
                    thunks.append(lambda w=which: t_group_open(w))
                    for k0 in range(0, NKT, 4):
                        thunks.append(lambda w=which, a=k0: t_mms(w, a, a + 4))
                    if which < 2:
                        thunks.append(lambda w=which: t_evict_qk(w))
                thunks.append(t_evict_v)
                for sc in range(ST // 128):
                    thunks.append(lambda s=sc: t_vtr(s))
                return thunks

            def qkv_stile(h, st):
                for t in qkv_stile_thunks(h, st):
                    t()

            LAG = 4
            GPS_KPS = {2, 4, 6, 8, 10, 12}  # denominator adds handled by GpSimd

            def attn_iter(h, qt, fillers=()):
                """One flash-attention iteration: 512 queries x full S keys.
                `fillers` are extra emission thunks interleaved between k-chunks."""
                fillers = list(fillers)
                fi = 0
                qsl = bass.ts(qt, QTL)
                oacc = psop.tile([128, QTL], F32, tag="oacc", name=f"oacc_{h}_{qt}")
                accA = accAp.tile([128, 2, QTL], FP16, tag="accA", name=f"accA_{h}_{qt}")
                accB = accBp.tile([128, 2, QTL], FP16, tag="accB", name=f"accB_{h}_{qt}")
                pts = {}
                for kc in range(NKC + LAG):
                    if kc < NKC:
                        pss = pssp.tile([128, QTL], F32, tag="pss",
                                        name=f"pss_{qt}_{h}_{kc}")
                        nc.tensor.matmul(
                            pss[:], KT[h][:, bass.ts(kc, 128)], QT[h][:, qsl],
                            start=True, stop=True,
                        )
                        kp = kc // 2
                        if kc % 2 == 0:
                            pts[kp] = ptp.tile([128, 2, QTL], FP16, tag="pt",
                                               name=f"pt_{qt}_{h}_{kp}")
                        nc.scalar.activation(
                            pts[kp][:, kc % 2, :], pss[:],
                            mybir.ActivationFunctionType.Exp, scale=SCALE,
                        )
                    while fi < len(fillers) and fi * (NKC + LAG) <= (kc + 1) * len(fillers):
                        fillers[fi]()
                        fi += 1
                    kd = kc - LAG
                    if 0 <= kd < NKC:
                        kp = kd // 2
                        nc.tensor.matmul(
                            oacc[:], V[h][:, kd, :], pts[kp][:, kd % 2, :],
                            start=(kd == 0), stop=(kd == NKC - 1),
                        )
                        if kd % 2 == 1:
                            # denominator partial sums, split DVE / GpSimd
                            if kp == 0:
                                nc.vector.tensor_copy(accA[:], pts[kp][:])
                            elif kp == 1:
                                nc.vector.tensor_copy(accB[:], pts[kp][:])
                            elif kp in GPS_KPS:
                                nc.gpsimd.tensor_add(accB[:], accB[:], pts[kp][:])
                            else:
                                nc.vector.tensor_add(accA[:], accA[:], pts[kp][:])
                            del pts[kp]
                while fi < len(fillers):
                    fillers[fi]()
                    fi += 1
                # denominator: all-ones matmul -> broadcast across partitions
                dn = psdp.tile([128, QTL], F32, tag="dn", name=f"dn_{h}_{qt}")
                nc.tensor.matmul(dn[:], ones_r[:], accA[:, 0, :], start=True, stop=False)
                nc.tensor.matmul(dn[:], ones_r[:], accA[:, 1, :], start=False, stop=False)
                nc.tensor.matmul(dn[:], ones_r[:], accB[:, 0, :], start=False, stop=False)
                nc.tensor.matmul(dn[:], ones_r[:], accB[:, 1, :], start=False, stop=True)
                rc = rcp.tile([128, QTL], F32, tag="rc", name=f"rc_{h}_{qt}")
                scr = rcp.tile([128, QTL], F32, tag="rcscr", name=f"rs_{h}_{qt}")
                nc.vector.reciprocal_approx_accurate(rc[:], dn[:], scr[:])
                nc.vector.tensor_mul(AT[h][:, qsl], oacc[:], rc[:])

            def outproj_qt(qt):
                """Output projection for this q-tile's 4 s-chunks (both heads ready)."""
                for sc4 in range(QTL // 128):
                    ssl = bass.ds(qt * QTL + sc4 * 128, 128)
                    for et in range(NET):
                        esl = bass.ts(et, ETL)
                        psy = ps1.tile([128, ETL], F32, tag="psqkv",
                                       name=f"psy_{qt}_{sc4}_{et}")
                        for h in range(HPC):
                            nc.tensor.matmul(
                                psy[:], AT[h][:, ssl], wo_sb[:, h, esl],
                                start=(h == 0), stop=(h == HPC - 1),
                            )
                        yt = yp.tile([128, ETL], F32, tag="yt", name=f"yt_{qt}_{sc4}_{et}")
                        nc.vector.tensor_copy(yt[:], psy[:])
                        nc.sync.dma_start(y_d[ssl, esl], yt[:])

            def outproj_thunks(qt):
                thunks = []
                for sc4 in range(QTL // 128):
                    for et in range(NET):
                        def blk(sc4=sc4, et=et):
                            ssl = bass.ds(qt * QTL + sc4 * 128, 128)
                            esl = bass.ts(et, ETL)
                            psy = ps1.tile([128, ETL], F32, tag="psqkv",
                                           name=f"psy_{qt}_{sc4}_{et}")
                            for h in range(HPC):
                                nc.tensor.matmul(
                                    psy[:], AT[h][:, ssl], wo_sb[:, h, esl],
                                    start=(h == 0), stop=(h == HPC - 1),
                                )
                            yt = yp.tile([128, ETL], F32, tag="yt",
                                         name=f"yt_{qt}_{sc4}_{et}")
                            nc.vector.tensor_copy(yt[:], psy[:])
                            nc.sync.dma_start(y_d[ssl, esl], yt[:])
                        thunks.append(blk)
                return thunks

            # stage A: QKV head 0
            for st in range(NST):
                qkv_stile(0, st)
            # stage B: attention(head0) || QKV head 1
            nc.gpsimd.dma_start(wo_sb[:], wo_r)
            for qt in range(NQT):
                attn_iter(0, qt)
                qkv_stile(1, qt)
            # stage C: attention(head1) || output projection
            for qt in range(NQT):
                attn_iter(1, qt)
                outproj_qt(qt)

    nc.compile()
    return nc


def _host_prep(x, w_qkv, b_qkv, w_out, S):
    """Build per-core input maps."""
    xT = np.ascontiguousarray(x.reshape(S, D).T).astype(np.float16)

    # RoPE tables (match reference._rope_cos_sin)
    inv_freq = (1.0 / (10000.0 ** (np.arange(0, ROT, 2, dtype=np.float32) / ROT))).astype(np.float32)
    t = np.arange(S, dtype=np.float32)
    freqs = np.outer(t, inv_freq)                      # [S, ROT/2]
    emb = np.concatenate([freqs, freqs], axis=-1)      # [S, ROT]
    cosT = np.cos(emb).astype(np.float32).T            # [ROT, S]
    sinT = np.sin(emb).astype(np.float32).T
    sinS = sinT.copy()
    sinS[0 : ROT // 2] *= -1.0
    cs = np.ascontiguousarray(np.stack([cosT, sinS], axis=1)).astype(np.float16)  # [ROT, 2, S]

    in_maps = []
    for c in range(NCORES):
        cols = []
        bcols = []
        for h in [HPC * c + i for i in range(HPC)]:
            for part in range(3):  # q, k, v
                off = part * D + h * HD
                cols.append(w_qkv[:, off : off + HD])
                bcols.append(b_qkv[off : off + HD])
        wsl = np.ascontiguousarray(np.stack(cols, axis=1)).astype(np.float16)   # [D, 3*HPC, 128]
        bsl = np.ascontiguousarray(np.stack(bcols, axis=1)).astype(np.float32)  # [128, 3*HPC]
        wout_sl = np.ascontiguousarray(w_out[c * HPC * HD : (c + 1) * HPC * HD, :]).astype(np.float16)
        in_maps.append({"xT": xT, "wsl": wsl, "bsl": bsl, "wout": wout_sl, "cs": cs})
    return in_maps


def kernel(x, w_qkv, b_qkv, w_out, b_out):
    B, S, D_ = x.shape
    assert B == 1 and D_ == D
    if "nc" not in _CACHE:
        _CACHE["nc"] = build_module(S=S)
    nc = _CACHE["nc"]
    in_maps = _host_prep(np.asarray(x, dtype=np.float32), np.asarray(w_qkv, dtype=np.float32),
                         np.asarray(b_qkv, dtype=np.float32), np.asarray(w_out, dtype=np.float32), S)
    res = run_bass_kernel_spmd(nc, in_maps, list(range(NCORES)))
    y = np.zeros((S, D), dtype=np.float32)
    for c in range(NCORES):
        y += res.results[c]["y"]
    y += np.asarray(b_out, dtype=np.float32)[None, :]
    return y.reshape(1, S, D)
